# revision 1
# baseline (speedup 1.0000x reference)
"""Trainium2 Bass kernel for CrossAttention (B=4, L=2048, D=1024, 16 heads x 64).

Sharding: 8 cores = 4 batches x 2 head-halves (tensor parallel over heads,
per the sharding hint: Wq/Wkv column-split, Wo row-split).  Per core:
Q = x1 @ Wq[:, half], K/V = x2 @ Wkv[:, half-cols], 8 heads of attention,
partial Y^T = Wo[half-rows]^T @ O^T.  The host transposes x per batch during
sharding (fp32 transposing DMAs are ~30x slower than contiguous and the xbar
DMA-transpose is 2-byte only), then sums the two partial Y^T per batch,
transposes back, and adds the bias.

Dataflow (per core), everything feature-major ("transposed") so the softmax
denominator reduction lands on the matmul contraction axis and no on-chip
transpose is ever needed:
  x^T tiles [D-part, L]      contiguous DMA from host-transposed x
  Q^T = Wq^T x1^T [IH, L]    lhsT = Wq tile (as stored), rhs = x1^T
  K^T = Wk^T x2^T [IH, L]
  V   = x2 @ Wv   [L, IH]    lhsT = x1^T tile, rhs = Wv tile (row-major)
  S^T tiles = lhsT K^T_h [64,128] x rhs Q^T_h [64,512] -> PSUM [128,512];
      the two heads of a pair ride PE row strips 0-63/64-127 concurrently
      (tile_position derived from base_partition)
  E^T = exp(S^T * 0.125)     ACT engine, PSUM -> SBUF, no max-subtraction
      needed (scores are N(0,1); exp is safe in fp32)
  U_aug += [V_h | 1x32]^T-matmul over j-tiles: rows 0-63 = unnormalized O^T,
      rows 64-95 = 32 copies of the softmax denominator
  O^T_h = U_h * recip(denom): all-DVE (reciprocal of the 32 denominator
      rows, stream_shuffle quadrant broadcast to 64 partitions, fused
      multiply evicts to SBUF)
  Y^T += Wo_tile^T @ O^T     accumulated over the inner half; each block's
      projection is deferred into the next block's attention to fill PE gaps

All matmuls run in float32r (full PE rate at moving free dim >= 256,
~tf32-precision multiplies): measured end-to-end relative error 3.1e-4.
Measured device time ~464 us/core (512-iteration device loop, wall delta);
cost-model timeline predicts 410 us. Engine busy: PE 336 us, ACT (exp
floor) 267 us, DVE 140 us, DMA 93 us.
"""

import numpy as np

# ---- problem constants (hardcoded; kernel.py must be self-contained) ----
B = 4
L = 2048
D = 1024
INNER = 1024
HEADS = 16
DH = 64
N_CORES = 8
IH = INNER // 2  # inner columns per core (8 heads)
SCALE = DH ** -0.5

_CACHE = {}


def _build_nc(L_=L, D_=D, IH_=IH, DH_=DH, LQB=512, XS=512, NOC=32, compile_=True, repeat=1):
    import concourse.bass as bass
    import concourse.tile as tile
    from concourse import bacc, mybir

    f32 = mybir.dt.float32
    f32r = mybir.dt.float32r
    P = 128
    NH = IH_ // DH_        # heads per core
    NPAIR = NH // 2        # head pairs
    NJT = L_ // P          # lk tiles
    NLQB = L_ // LQB       # lq blocks
    NDT = D_ // P          # D tiles
    NIT = IH_ // P         # inner-half tiles
    NXS = L_ // XS         # x slices
    assert NH % 2 == 0 and L_ % LQB == 0 and LQB % P == 0

    nc = bacc.Bacc("TRN2", target_bir_lowering=False, debug=False)
    x1 = nc.declare_dram_parameter("x1t", [D_, L_], f32r, isOutput=False)
    x2 = nc.declare_dram_parameter("x2t", [D_, L_], f32r, isOutput=False)
    wq = nc.declare_dram_parameter("wq", [D_, IH_], f32r, isOutput=False)
    wk = nc.declare_dram_parameter("wk", [D_, IH_], f32r, isOutput=False)
    wv = nc.declare_dram_parameter("wv", [D_, IH_], f32r, isOutput=False)
    wo = nc.declare_dram_parameter("wo", [IH_, D_], f32r, isOutput=False)
    yt = nc.declare_dram_parameter("yt", [D_, L_], f32, isOutput=True)

    import contextlib

    with tile.TileContext(nc) as tc:
        with (
            tc.For_i(0, repeat, 1) if repeat > 1 else contextlib.nullcontext(),
            tc.tile_pool(name="persist", bufs=1) as persist,
        ):
            # persistent SBUF tensors (one slot each; distinct names)
            kt = persist.tile([P, NIT * L_], f32r, name="kt")    # K^T blocks
            onesrow = persist.tile([P, NH * NOC], f32, name="onesrow")
            nc.vector.memset(onesrow, 1.0)
            qt = persist.tile([P, NIT * L_], f32r, name="qt")    # Q^T blocks
            vv = persist.tile([P, NJT * NH * (DH_ + NOC)], f32r, name="vv")
            VJ = NH * (DH_ + NOC)  # per-j-tile v width

            # ---- phases 1+2: projections ----
            with (
                tc.tile_pool(name="wts", bufs=1) as wts,
                tc.tile_pool(name="xts", bufs=2) as xts,
                tc.tile_pool(name="psum_acc", bufs=1, space="PSUM") as acc_pool,
            ):
                wkt = wts.tile([P, NDT * IH_], f32r, name="wkt")
                wvt = wts.tile([P, NDT * IH_], f32r, name="wvt")
                wqt = wts.tile([P, NDT * IH_], f32r, name="wqt")

                def load_weights():
                    for d in range(NDT):
                        if d > 0:
                            nc.sync.dma_start(
                                out=wkt[:, d * IH_:(d + 1) * IH_],
                                in_=wk[d * P:(d + 1) * P, :])
                        nc.sync.dma_start(
                            out=wvt[:, d * IH_:(d + 1) * IH_],
                            in_=wv[d * P:(d + 1) * P, :])
                        nc.sync.dma_start(
                            out=wqt[:, d * IH_:(d + 1) * IH_],
                            in_=wq[d * P:(d + 1) * P, :])

                # ones columns of the V_aug layout (DVE cast-copy from the
                # f32 ones tile; memset cannot write f32r directly)
                for j in range(NJT):
                    ones_ap = (vv[:, j * VJ:(j + 1) * VJ]
                               .rearrange("p (h c) -> p h c", c=DH_ + NOC)[:, :, DH_:])
                    nc.vector.tensor_copy(ones_ap, onesrow.rearrange(
                        "p (h c) -> p h c", c=NOC))

                def load_xt_slice(xt_dram, s):
                    tiles = []
                    for d in range(NDT):
                        xt_t = xts.tile([P, XS], f32r, name="xt",
                                        tag=f"xt{d}",
                                        bufs=2 if d < NDT // 2 else 1)
                        nc.sync.dma_start(
                            out=xt_t,
                            in_=xt_dram[d * P:(d + 1) * P, s * XS:(s + 1) * XS])
                        tiles.append(xt_t)
                    return tiles

                # phase 1: K^T and V from x2 (first weight tile, then x
                # slices, then the remaining weights, so PE starts as early
                # as possible). d is the outer loop so each xt[d] tile is
                # consumed the moment its DMA lands.
                nc.sync.dma_start(out=wkt[:, 0:IH_], in_=wk[0:P, :])
                for s in range(NXS):
                    xt = load_xt_slice(x2, s)
                    if s == 0:
                        load_weights()
                    pks = [acc_pool.tile([P, XS], f32, name="pk",
                                         tag=f"pk{m}", bufs=1)
                           for m in range(NIT)]
                    pvs = [acc_pool.tile([P, IH_], f32, name="pv",
                                         tag=f"pv{t}", bufs=1)
                           for t in range(XS // P)]
                    for d in range(NDT):
                        for m in range(NIT):
                            nc.tensor.matmul(
                                pks[m],
                                lhsT=wkt[:, d * IH_ + m * P: d * IH_ + (m + 1) * P],
                                rhs=xt[d],
                                start=(d == 0), stop=(d == NDT - 1))
                        for t in range(XS // P):
                            nc.tensor.matmul(
                                pvs[t],
                                lhsT=xt[d][:, t * P:(t + 1) * P],
                                rhs=wvt[:, d * IH_:(d + 1) * IH_],
                                start=(d == 0), stop=(d == NDT - 1))
                    for m in range(NIT):
                        nc.vector.tensor_copy(
                            kt[:, m * L_ + s * XS: m * L_ + (s + 1) * XS],
                            pks[m])
                    for t in range(XS // P):
                        j = s * (XS // P) + t
                        dst = (vv[:, j * VJ:(j + 1) * VJ]
                               .rearrange("p (h c) -> p h c", c=DH_ + NOC)[:, :, :DH_])
                        srcv = pvs[t].rearrange("p (h c) -> p h c", c=DH_)
                        nc.vector.tensor_copy(dst, srcv)

                # phase 2: Q^T from x1
                for s in range(NXS):
                    xt = load_xt_slice(x1, s)
                    pqs = [acc_pool.tile([P, XS], f32, name="pq",
                                         tag=f"pk{m}", bufs=1)
                           for m in range(NIT)]
                    for d in range(NDT):
                        for m in range(NIT):
                            nc.tensor.matmul(
                                pqs[m],
                                lhsT=wqt[:, d * IH_ + m * P: d * IH_ + (m + 1) * P],
                                rhs=xt[d],
                                start=(d == 0), stop=(d == NDT - 1))
                    for m in range(NIT):
                        nc.vector.tensor_copy(
                            qt[:, m * L_ + s * XS: m * L_ + (s + 1) * XS],
                            pqs[m])

            # ---- phase 3: attention + output projection ----
            with (
                tc.tile_pool(name="wo_pool", bufs=1) as wo_pool,
                tc.tile_pool(name="spsum", bufs=2, space="PSUM") as s_pool,
                tc.tile_pool(name="upsum", bufs=4, space="PSUM") as u_pool,
                tc.tile_pool(name="ets", bufs=3) as ets,
                tc.tile_pool(name="smalls", bufs=4) as smalls,
                tc.tile_pool(name="ot_pool", bufs=2) as ot_pool,
                tc.tile_pool(name="youts", bufs=3) as youts,
            ):
                wot = wo_pool.tile([P, NIT * D_], f32r, name="wot")
                for it in range(NIT):
                    nc.sync.dma_start(
                        out=wot[:, it * D_:(it + 1) * D_],
                        in_=wo[it * P:(it + 1) * P, :])

                def emit_yproj(ot_prev, lqb_prev, dsub):
                    py = u_pool.tile([P, LQB], f32, name="py", tag="u")
                    for it in range(NIT):
                        nc.tensor.matmul(
                            py,
                            lhsT=wot[:, it * D_ + dsub * P: it * D_ + (dsub + 1) * P],
                            rhs=ot_prev[:, it * LQB:(it + 1) * LQB],
                            start=(it == 0), stop=(it == NIT - 1))
                    yo = youts.tile([P, LQB], f32, name="yo")
                    nc.vector.tensor_copy(yo, py)
                    nc.sync.dma_start(
                        out=yt[dsub * P:(dsub + 1) * P,
                               lqb_prev * LQB:(lqb_prev + 1) * LQB],
                        in_=yo)

                ydefer = []  # (ot, lqb, dsub) of the previous block

                for lqb in range(NLQB):
                    ot = ot_pool.tile([P, NIT * LQB], f32r, name="ot")
                    for hp in range(NPAIR):
                        u0 = u_pool.tile([P, LQB], f32, name="u0", tag="u")
                        u1 = u_pool.tile([P, LQB], f32, name="u1", tag="u")
                        for j in range(NJT):
                            st = s_pool.tile([P, 2 * LQB], f32, name="st",
                                             tag="st")
                            # head pair rides PE rows 0-63 / 64-127
                            nc.tensor.matmul(
                                st[:, 0:LQB],
                                lhsT=kt[0:DH_, hp * L_ + j * P: hp * L_ + (j + 1) * P],
                                rhs=qt[0:DH_, hp * L_ + lqb * LQB: hp * L_ + lqb * LQB + LQB],
                                start=True, stop=True)
                            nc.tensor.matmul(
                                st[:, LQB:2 * LQB],
                                lhsT=kt[DH_:2 * DH_, hp * L_ + j * P: hp * L_ + (j + 1) * P],
                                rhs=qt[DH_:2 * DH_, hp * L_ + lqb * LQB: hp * L_ + lqb * LQB + LQB],
                                start=True, stop=True)
                            et = ets.tile([P, 2 * LQB], f32r, name="et")
                            nc.scalar.activation(
                                et, st, mybir.ActivationFunctionType.Exp,
                                scale=float(SCALE))
                            for hh, u in ((0, u0), (1, u1)):
                                h = 2 * hp + hh
                                nc.tensor.matmul(
                                    u[0:DH_ + NOC, :],
                                    lhsT=vv[:, j * VJ + h * (DH_ + NOC): j * VJ + (h + 1) * (DH_ + NOC)],
                                    rhs=et[:, hh * LQB:(hh + 1) * LQB],
                                    start=(j == 0), stop=(j == NJT - 1))
                        for hh, u in ((0, u0), (1, u1)):
                            # all-DVE normalization: recip of the (replicated)
                            # denominator rows, quadrant-broadcast via
                            # stream_shuffle, fused multiply evicts O^T
                            rcp = smalls.tile([NOC, LQB], f32, name="rcp",
                                              tag=f"rcp{hh}")
                            nc.vector.reciprocal(rcp, u[DH_:DH_ + NOC, :])
                            rb = smalls.tile([DH_, LQB], f32, name="rb",
                                             tag=f"rb{hh}")
                            zmask = [0] * 32
                            nc.vector.stream_shuffle(rb[0:32, :], rcp, zmask)
                            nc.vector.stream_shuffle(rb[32:64, :], rcp, zmask)
                            dst = ot[hh * DH_:(hh + 1) * DH_,
                                     hp * LQB:(hp + 1) * LQB]
                            nc.vector.tensor_mul(dst, u[0:DH_, :], rb)
                        # two output-projection column groups of the previous
                        # lq block ride along to fill PE gaps
                        for _ in range(3):
                            if ydefer:
                                emit_yproj(*ydefer.pop(0))
                    ydefer.extend((ot, lqb, dsub) for dsub in range(D_ // P))
                while ydefer:
                    emit_yproj(*ydefer.pop(0))
    if compile_:
        nc.compile()
    return nc


def _get_nc():
    if "nc" not in _CACHE:
        _CACHE["nc"] = _build_nc()
    return _CACHE["nc"]


def kernel(x1, x2, Wq, Wkv, Wo, bo):
    import sys
    if "/opt/trn_rl_repo" not in sys.path:
        sys.path.insert(0, "/opt/trn_rl_repo")
    from concourse.bass_utils import run_bass_kernel_spmd

    x1 = np.asarray(x1, dtype=np.float32)
    x2 = np.asarray(x2, dtype=np.float32)
    Wq = np.asarray(Wq, dtype=np.float32)
    Wkv = np.asarray(Wkv, dtype=np.float32)
    Wo = np.asarray(Wo, dtype=np.float32)
    bo = np.asarray(bo, dtype=np.float32)

    nc = _get_nc()
    res = run_bass_kernel_spmd(nc, _make_in_maps(x1, x2, Wq, Wkv, Wo),
                               list(range(N_CORES)))
    return _gather(res.results, bo)


def _make_in_maps(x1, x2, Wq, Wkv, Wo):
    x1t = [np.ascontiguousarray(x1[b].T) for b in range(B)]
    x2t = [np.ascontiguousarray(x2[b].T) for b in range(B)]
    in_maps = []
    for c in range(N_CORES):
        b, t = c // 2, c % 2
        in_maps.append({
            "x1t": x1t[b],
            "x2t": x2t[b],
            "wq": np.ascontiguousarray(Wq[:, t * IH:(t + 1) * IH]),
            "wk": np.ascontiguousarray(Wkv[:, t * IH:(t + 1) * IH]),
            "wv": np.ascontiguousarray(Wkv[:, INNER + t * IH: INNER + (t + 1) * IH]),
            "wo": np.ascontiguousarray(Wo[t * IH:(t + 1) * IH, :]),
        })
    return in_maps


def _gather(outs, bo):
    y = np.empty((B, L, D), dtype=np.float32)
    for b in range(B):
        y[b] = (outs[2 * b]["yt"] + outs[2 * b + 1]["yt"]).T + bo
    return y



# revision 30
# speedup vs baseline: 1.1556x; 1.1556x over previous
"""Trainium2 Bass kernel for CrossAttention (B=4, L=2048, D=1024, 16 heads x 64).

Sharding: 8 cores = 4 batches x 2 head-halves (tensor parallel over heads:
Wq/Wkv column-split, Wo row-split).  Per core: Q = x1 @ Wq[:, half],
K/V = x2 @ Wkv[:, half-cols], 8 heads of attention, partial Y^T =
Wo[half-rows]^T @ O^T.  The host transposes x per batch (and casts x/Wq/Wkv
to bf16) during sharding, sums the two partial Y^T per batch, transposes
back, adds the bias.

v6 schedule: there is no projection "phase" -- every projection is a
single-PSUM-bank accumulate-and-copy *job* (K per (m-group, x-slice), V per
(x-slice, quarter), Q per (slice, m-group), Y^T per (block, D-subtile)),
and the jobs are woven into the attention group loop.  Attention block 0
absorbs the whole K/V/Q production, so the exp stream on ACT (265.7us busy,
the #2 engine after PE's 327.7us) starts ~10us in instead of ~70us, and the
schedule is PE-bound end to end.  The j-loop runs in groups of 2 j-tiles
with U lagging TWO groups behind S: after exp(j)'s sem fires, S(j+2)
outranks U(j) in the Tile scheduler (earlier emission), keeping the exp
stream dense; U's and jobs fill the remaining PE time.  The S->exp->
st-slot-recycle loop is the only tight coupling (st bufs=2).

dtypes: x1/x2/Wq/Wk/Wv and K^T/Q^T/V/E in bf16 (matmul rate identical to
f32r; halves SBUF+DMA so all four x2 slices stay resident for the m-major
K jobs); S stays f32 in PSUM; O^T/Wo/Y^T in f32r.  Measured rel err ~2e-3.

PSUM: st 2x[128,1024] (4 banks) + u 2x[128,512] + jobs 2x[128,512] = 8.
All matmuls 1 row/cycle (moving free dim 512 >= 256).
"""

import numpy as np

# ---- problem constants (hardcoded; kernel.py must be self-contained) ----
B = 4
L = 2048
D = 1024
INNER = 1024
HEADS = 16
DH = 64
N_CORES = 8
IH = INNER // 2  # inner columns per core (8 heads)
SCALE = DH ** -0.5

_CACHE = {}


def _build_nc(L_=L, D_=D, IH_=IH, DH_=DH, LQB=512, XS=512, NOC=64, compile_=True, repeat=1):
    import concourse.bass as bass
    import concourse.tile as tile
    from concourse import bacc, mybir

    f32 = mybir.dt.float32
    f32r = mybir.dt.float32r
    bf16 = mybir.dt.bfloat16
    P = 128
    NH = IH_ // DH_        # heads per core
    NPAIR = NH // 2        # head pairs
    NJT = L_ // P          # lk tiles
    NLQB = L_ // LQB       # lq blocks
    NDT = D_ // P          # D tiles
    NIT = IH_ // P         # inner-half tiles
    NXS = L_ // XS         # x slices
    assert NH % 2 == 0 and L_ % LQB == 0 and LQB % P == 0 and XS == LQB

    # host-packed layouts: one contiguous DMA per weight / per x-slice
    # (HWDGE issue overhead is 625ns per dma_start; 64 tile-DMAs starve the
    # early K jobs).  xh[p, s*D + d*XS + c] = x^T[d*128+p, s*XS+c];
    # wh[p, d*IH + c] = w[d*128+p, c]; woh[p, it*D + c] = wo[it*128+p, c].
    nc = bacc.Bacc("TRN2", target_bir_lowering=False, debug=False)
    x1 = nc.declare_dram_parameter("x1h", [P, L_ * D_ // P], bf16, isOutput=False)
    x2 = nc.declare_dram_parameter("x2h", [P, L_ * D_ // P], bf16, isOutput=False)
    wq = nc.declare_dram_parameter("wqh", [P, D_ * IH_ // P], bf16, isOutput=False)
    wk = nc.declare_dram_parameter("wkh", [P, D_ * IH_ // P], bf16, isOutput=False)
    wv = nc.declare_dram_parameter("wvh", [P, D_ * IH_ // P], bf16, isOutput=False)
    wo = nc.declare_dram_parameter("woh", [P, IH_ * D_ // P], f32r, isOutput=False)
    yt = nc.declare_dram_parameter("yt", [D_, L_], f32, isOutput=True)

    import contextlib

    with tile.TileContext(nc) as tc:
        with (
            tc.For_i(0, repeat, 1) if repeat > 1 else contextlib.nullcontext(),
            tc.tile_pool(name="persist", bufs=1) as persist,
        ):
            # persistent SBUF tensors (one slot each; distinct names)
            kt = persist.tile([P, NIT * L_], bf16, name="kt")    # K^T blocks
            qt = persist.tile([P, NIT * L_], bf16, name="qt")    # Q^T blocks
            vv = persist.tile([P, NJT * NH * (DH_ + NOC)], bf16, name="vv")
            onesrow = persist.tile([P, NH * NOC], f32, name="onesrow")
            wkt = persist.tile([P, NDT * IH_], bf16, name="wkt")
            wvt = persist.tile([P, NDT * IH_], bf16, name="wvt")
            wqt = persist.tile([P, NDT * IH_], bf16, name="wqt")
            wot = persist.tile([P, NIT * D_], f32r, name="wot")
            nc.vector.memset(onesrow, 1.0)
            VJ = NH * (DH_ + NOC)  # per-j-tile v width

            with (
                tc.tile_pool(name="xts2", bufs=1) as xts2,
                tc.tile_pool(name="xts1", bufs=2) as xts1,
                tc.tile_pool(name="jobs", bufs=2, space="PSUM") as jobs,
                tc.tile_pool(name="spsum", bufs=2, space="PSUM") as s_pool,
                tc.tile_pool(name="upsum", bufs=2, space="PSUM") as u_pool,
                tc.tile_pool(name="ets", bufs=6) as ets,
                tc.tile_pool(name="smalls", bufs=1) as smalls,
                tc.tile_pool(name="ot_pool", bufs=2) as ot_pool,
                tc.tile_pool(name="youts", bufs=3) as youts,
            ):
                # ones columns of the V_aug layout (DVE cast-copy to bf16)
                for j in range(NJT):
                    ones_ap = (vv[:, j * VJ:(j + 1) * VJ]
                               .rearrange("p (h c) -> p h c", c=DH_ + NOC)[:, :, DH_:])
                    nc.vector.tensor_copy(ones_ap, onesrow.rearrange(
                        "p (h c) -> p h c", c=NOC))

                # ---- DMA issue: one contiguous transfer per weight/slice,
                # deadline-ordered on the in-order DMA queue ----
                xt2 = {}   # s -> slice tile [P, NDT*XS]; d-tile = col view
                xt1s = {}

                def load_x2_slice(s):
                    t = xts2.tile([P, NDT * XS], bf16, name="x2t",
                                  tag=f"x2_{s}", bufs=1)
                    nc.sync.dma_start(
                        out=t, in_=x2[:, s * NDT * XS:(s + 1) * NDT * XS])
                    xt2[s] = t

                def load_x1_slice(sl):
                    t = xts1.tile([P, NDT * XS], bf16, name="x1t",
                                  tag="x1", bufs=2)
                    nc.sync.dma_start(
                        out=t, in_=x1[:, sl * NDT * XS:(sl + 1) * NDT * XS])
                    xt1s[sl] = t

                # half-transfers up front: Tile's range-based dependency
                # tracking lets the d0-3 matmuls of the first K/Q jobs start
                # after half a slice has landed
                MW = NDT * P  # one m-group of a (m-major) weight
                HX = NDT * XS // 2
                nc.sync.dma_start(out=wkt[:, 0:MW], in_=wk[:, 0:MW])
                t0 = xts2.tile([P, NDT * XS], bf16, name="x2t", tag="x2_0",
                               bufs=1)
                xt2[0] = t0
                nc.sync.dma_start(out=t0[:, 0:HX], in_=x2[:, 0:HX])
                nc.sync.dma_start(out=t0[:, HX:], in_=x2[:, HX:2 * HX])
                nc.sync.dma_start(out=wqt[:, 0:MW], in_=wq[:, 0:MW])
                t1 = xts1.tile([P, NDT * XS], bf16, name="x1t", tag="x1",
                               bufs=2)
                xt1s[0] = t1
                nc.sync.dma_start(out=t1[:, 0:HX], in_=x1[:, 0:HX])
                nc.sync.dma_start(out=t1[:, HX:], in_=x1[:, HX:2 * HX])
                nc.sync.dma_start(out=wkt[:, MW:], in_=wk[:, MW:NIT * MW])
                nc.sync.dma_start(out=wvt, in_=wv[:, :])
                nc.sync.dma_start(out=wqt[:, MW:], in_=wq[:, MW:NIT * MW])
                for s in range(1, NXS):
                    load_x2_slice(s)
                nc.sync.dma_start(out=wot, in_=wo[:, :])

                # ---- uniform single-bank PSUM jobs ----
                def kjob(m, s):
                    t = jobs.tile([P, XS], f32, name="kj", tag="job")
                    for d in range(NDT):
                        nc.tensor.matmul(
                            t,
                            lhsT=wkt[:, (m * NDT + d) * P: (m * NDT + d + 1) * P],
                            rhs=xt2[s][:, d * XS:(d + 1) * XS],
                            start=(d == 0), stop=(d == NDT - 1))
                    nc.vector.tensor_copy(
                        kt[:, m * L_ + s * XS: m * L_ + (s + 1) * XS], t)

                def vjob(s, tq):
                    t = jobs.tile([P, IH_], f32, name="vj", tag="job")
                    for d in range(NDT):
                        nc.tensor.matmul(
                            t,
                            lhsT=xt2[s][:, d * XS + tq * P: d * XS + (tq + 1) * P],
                            rhs=wvt[:, d * IH_:(d + 1) * IH_],
                            start=(d == 0), stop=(d == NDT - 1))
                    j = s * (XS // P) + tq
                    dst = (vv[:, j * VJ:(j + 1) * VJ]
                           .rearrange("p (h c) -> p h c", c=DH_ + NOC)[:, :, :DH_])
                    nc.vector.tensor_copy(dst, t.rearrange("p (h c) -> p h c", c=DH_))

                def qjob(sl, m):
                    t = jobs.tile([P, XS], f32, name="qj", tag="job")
                    for d in range(NDT):
                        nc.tensor.matmul(
                            t,
                            lhsT=wqt[:, (m * NDT + d) * P: (m * NDT + d + 1) * P],
                            rhs=xt1s[sl][:, d * XS:(d + 1) * XS],
                            start=(d == 0), stop=(d == NDT - 1))
                    nc.vector.tensor_copy(
                        qt[:, m * L_ + sl * XS: m * L_ + (sl + 1) * XS], t)

                def yjob(ot_prev, lqb_prev, dsub, py=None, it0=0):
                    if py is None:
                        py = jobs.tile([P, LQB], f32, name="py", tag="job")
                    for it in range(it0, NIT):
                        nc.tensor.matmul(
                            py,
                            lhsT=wot[:, it * D_ + dsub * P: it * D_ + (dsub + 1) * P],
                            rhs=ot_prev[:, it * LQB:(it + 1) * LQB],
                            start=(it == 0), stop=(it == NIT - 1))
                    yo = youts.tile([P, LQB], f32, name="yo")
                    nc.vector.tensor_copy(yo, py)
                    nc.sync.dma_start(
                        out=yt[dsub * P:(dsub + 1) * P,
                               lqb_prev * LQB:(lqb_prev + 1) * LQB],
                        in_=yo)

                def yjob_partial(ot_prev, dsub):
                    # last block: accumulate the pair-0..2 contributions of
                    # Y^T before pair 3's normalization lands
                    py = jobs.tile([P, LQB], f32, name="pyp", tag="job")
                    for it in range(NIT - 1):
                        nc.tensor.matmul(
                            py,
                            lhsT=wot[:, it * D_ + dsub * P: it * D_ + (dsub + 1) * P],
                            rhs=ot_prev[:, it * LQB:(it + 1) * LQB],
                            start=(it == 0), stop=False)
                    return py

                def run_job(spec):
                    kind = spec[0]
                    if kind == "k":
                        kjob(spec[1], spec[2])
                    elif kind == "v":
                        vjob(spec[1], spec[2])
                    elif kind == "q":
                        qjob(spec[1], spec[2])

                # ---- attention helpers ----
                def emit_norm(hp, u0, u1, ot_b, last=False):
                    for hh, u in ((0, u0), (1, u1)):
                        # all-DVE normalization: recip of the (replicated)
                        # denominator rows, quadrant-broadcast via
                        # stream_shuffle, fused multiply evicts O^T
                        # NOC == DH: the reciprocal of the replicated
                        # denominator rows lines up 1:1 with the V rows --
                        # no broadcast needed
                        rcp = smalls.tile([NOC, LQB], f32, name="rcp",
                                          tag=f"rcp{hh}")
                        nc.vector.reciprocal(rcp, u[DH_:DH_ + NOC, :])
                        dst = ot_b[hh * DH_:(hh + 1) * DH_,
                                   hp * LQB:(hp + 1) * LQB]
                        nc.vector.tensor_mul(dst, u[0:DH_, :], rcp)

                def emit_u(hp, u0, u1, et, j, ot_b):
                    for hh, u in ((0, u0), (1, u1)):
                        h = 2 * hp + hh
                        nc.tensor.matmul(
                            u[0:DH_ + NOC, :],
                            lhsT=vv[:, j * VJ + h * (DH_ + NOC): j * VJ + (h + 1) * (DH_ + NOC)],
                            rhs=et[:, hh * LQB:(hh + 1) * LQB],
                            start=(j == 0), stop=(j == NJT - 1))
                    if j == NJT - 1:
                        emit_norm(hp, u0, u1, ot_b,
                                  last=(hp == NPAIR - 1 and ot_b is last_ot[0]))

                # block-0 insert schedule: all remaining K/V/Q jobs, placed so
                # each pair's S inputs (kt m-band, qt m-group) complete a pair
                # early, and V slices land roughly with their consuming U's
                # EMISSION order is dependency order in Tile: every job must
                # be emitted strictly before its first consumer (vjob(s,t)
                # before the U of j=4s+t, popped at group j//2+2 of pair 0;
                # kjob(m,s) before S(pair m, j=4s); qjob(0,m) before pair m)
                b0_ins = {
                    (0, 0): [("k", 0, 1), ("v", 1, 0)],
                    (0, 1): [("k", 0, 2), ("v", 1, 1)],
                    (0, 2): [("k", 0, 3), ("v", 1, 2), ("q", 0, 1)],
                    (0, 3): [("v", 1, 3), ("v", 2, 0)],
                    (0, 4): [("v", 2, 1), ("v", 2, 2)],
                    (0, 5): [("v", 2, 3), ("v", 3, 0)],
                    (0, 6): [("v", 3, 1), ("v", 3, 2)],
                    (0, 7): [("v", 3, 3), ("k", 1, 0)],
                    (1, 0): [("k", 1, 1), ("k", 1, 2)],
                    (1, 1): [("k", 1, 3)],
                    (1, 2): [("q", 0, 2)],
                    (1, 3): [("k", 2, 0)],
                    (1, 4): [("k", 2, 1), ("k", 2, 2)],
                    (1, 5): [("k", 2, 3)],
                    (2, 0): [("q", 0, 3)],
                    (2, 1): [("k", 3, 0), ("k", 3, 1)],
                    (2, 2): [("k", 3, 2), ("k", 3, 3)],
                    (2, 4): [("q", 1, 0)], (2, 6): [("q", 1, 1)],
                    (3, 1): [("q", 1, 2)], (3, 4): [("q", 1, 3)],
                }

                # prologue: just enough for pair 0's first S/exp/U
                kjob(0, 0)
                qjob(0, 0)
                for tq in range(XS // P):
                    vjob(0, tq)

                ydefer = []  # (ot, lqb, dsub) of the previous block
                pend = []    # (hp, u0, u1, et, j, ot) awaiting the U matmul;
                             # carried across pair/block edges so the last
                             # j-group's U never waits on its exp
                last_ot = [None]
                partials = []

                for lqb in range(NLQB):
                    if lqb < NLQB - 1:
                        load_x1_slice(lqb + 1)
                    ot = ot_pool.tile([P, NIT * LQB], f32r, name="ot")
                    if lqb == NLQB - 1:
                        last_ot[0] = ot
                    for hp in range(NPAIR):
                        u0 = u_pool.tile([P, LQB], f32, name="u0", tag="u")
                        u1 = u_pool.tile([P, LQB], f32, name="u1", tag="u")
                        for g in range(NJT // 2):
                            for jj in (2 * g, 2 * g + 1):
                                st = s_pool.tile([P, 2 * LQB], f32, name="st",
                                                 tag="st")
                                # head pair rides PE rows 0-63 / 64-127
                                nc.tensor.matmul(
                                    st[:, 0:LQB],
                                    lhsT=kt[0:DH_, hp * L_ + jj * P: hp * L_ + (jj + 1) * P],
                                    rhs=qt[0:DH_, hp * L_ + lqb * LQB: hp * L_ + lqb * LQB + LQB],
                                    start=True, stop=True)
                                nc.tensor.matmul(
                                    st[:, LQB:2 * LQB],
                                    lhsT=kt[DH_:2 * DH_, hp * L_ + jj * P: hp * L_ + (jj + 1) * P],
                                    rhs=qt[DH_:2 * DH_, hp * L_ + lqb * LQB: hp * L_ + lqb * LQB + LQB],
                                    start=True, stop=True)
                                et = ets.tile([P, 2 * LQB], bf16, name="et")
                                nc.scalar.activation(
                                    et, st, mybir.ActivationFunctionType.Exp,
                                    scale=float(SCALE))
                                pend.append((hp, u0, u1, et, jj, ot))
                            while len(pend) > 4:  # U lags 2 groups behind S
                                emit_u(*pend.pop(0))
                            if lqb == 0:
                                for spec in b0_ins.get((hp, g), ()):
                                    run_job(spec)
                            else:
                                if g in (2, 6) and ydefer:
                                    yjob(*ydefer.pop(0))
                                if g == 4 and lqb < NLQB - 1:
                                    qjob(lqb + 1, hp)
                                if (lqb == NLQB - 1 and hp == NPAIR - 1
                                        and g == 7):
                                    partials.append((yjob_partial(ot, 0), 0))
                                    partials.append((yjob_partial(ot, 1), 1))
                    skip = {0, 1} if lqb == NLQB - 1 else ()
                    ydefer.extend((ot, lqb, dsub)
                                  for dsub in range(D_ // P) if dsub not in skip)
                while pend:
                    emit_u(*pend.pop(0))
                # tail: last block's output projections (partials first)
                for py, dsub in partials:
                    yjob(last_ot[0], NLQB - 1, dsub, py=py, it0=NIT - 1)
                while ydefer:
                    yjob(*ydefer.pop(0))
    if compile_:
        nc.compile()
    return nc


def _get_nc():
    if "nc" not in _CACHE:
        _CACHE["nc"] = _build_nc()
    return _CACHE["nc"]


def kernel(x1, x2, Wq, Wkv, Wo, bo):
    import sys
    if "/opt/trn_rl_repo" not in sys.path:
        sys.path.insert(0, "/opt/trn_rl_repo")
    from concourse.bass_utils import run_bass_kernel_spmd

    x1 = np.asarray(x1, dtype=np.float32)
    x2 = np.asarray(x2, dtype=np.float32)
    Wq = np.asarray(Wq, dtype=np.float32)
    Wkv = np.asarray(Wkv, dtype=np.float32)
    Wo = np.asarray(Wo, dtype=np.float32)
    bo = np.asarray(bo, dtype=np.float32)

    nc = _get_nc()
    res = run_bass_kernel_spmd(nc, _make_in_maps(x1, x2, Wq, Wkv, Wo),
                               list(range(N_CORES)))
    return _gather(res.results, bo)


def _pack_x(xt, dt):
    # [D, L] -> [128, NXS * D] with xh[p, s*D + d*XS + c] = xt[d*128+p, s*XS+c]
    NXS, XS, NDT, P = 4, 512, 8, 128  # matches _build_nc defaults
    v = xt.reshape(NDT, P, NXS, XS).transpose(1, 2, 0, 3).reshape(P, NXS * NDT * XS)
    return np.ascontiguousarray(v).astype(dt)


def _pack_w(w, dt):
    # [K, N] -> [128, (K//128) * N] with wh[p, d*N + c] = w[d*128+p, c]
    K, N = w.shape
    v = w.reshape(K // 128, 128, N).transpose(1, 0, 2).reshape(128, (K // 128) * N)
    return np.ascontiguousarray(v).astype(dt)


def _pack_w_mmajor(w, dt):
    # [K, N] -> [128, (K//128) * N] with wh[p, (m*(K//128)+d)*128 + c] =
    # w[d*128+p, m*128+c]: each m-group's weights are contiguous
    K, N = w.shape
    v = (w.reshape(K // 128, 128, N // 128, 128)      # d p m c
         .transpose(1, 2, 0, 3)                        # p m d c
         .reshape(128, (K // 128) * N))
    return np.ascontiguousarray(v).astype(dt)


def _make_in_maps(x1, x2, Wq, Wkv, Wo):
    import ml_dtypes
    bf = ml_dtypes.bfloat16
    x1h = [_pack_x(x1[b].T, bf) for b in range(B)]
    x2h = [_pack_x(x2[b].T, bf) for b in range(B)]
    in_maps = []
    for c in range(N_CORES):
        b, t = c // 2, c % 2
        in_maps.append({
            "x1h": x1h[b],
            "x2h": x2h[b],
            "wqh": _pack_w_mmajor(Wq[:, t * IH:(t + 1) * IH], bf),
            "wkh": _pack_w_mmajor(Wkv[:, t * IH:(t + 1) * IH], bf),
            "wvh": _pack_w(Wkv[:, INNER + t * IH: INNER + (t + 1) * IH], bf),
            "woh": _pack_w(Wo[t * IH:(t + 1) * IH, :], np.float32),
        })
    return in_maps


def _gather(outs, bo):
    y = np.empty((B, L, D), dtype=np.float32)
    for b in range(B):
        y[b] = (outs[2 * b]["yt"] + outs[2 * b + 1]["yt"]).T + bo
    return y


# revision 32
# speedup vs baseline: 1.1620x; 1.0055x over previous
"""Trainium2 Bass kernel for CrossAttention (B=4, L=2048, D=1024, 16 heads x 64).

Sharding: 8 cores = 4 batches x 2 head-halves (tensor parallel over heads:
Wq/Wkv column-split, Wo row-split).  Per core: Q = x1 @ Wq[:, half],
K/V = x2 @ Wkv[:, half-cols], 8 heads of attention, partial Y^T =
Wo[half-rows]^T @ O^T.  The host transposes x per batch (and casts x/Wq/Wkv
to bf16) during sharding, sums the two partial Y^T per batch, transposes
back, adds the bias.

v6 schedule: there is no projection "phase" -- every projection is a
single-PSUM-bank accumulate-and-copy *job* (K per (m-group, x-slice), V per
(x-slice, quarter), Q per (slice, m-group), Y^T per (block, D-subtile)),
and the jobs are woven into the attention group loop.  Attention block 0
absorbs the whole K/V/Q production, so the exp stream on ACT (265.7us busy,
the #2 engine after PE's 327.7us) starts ~10us in instead of ~70us, and the
schedule is PE-bound end to end.  The j-loop runs in groups of 2 j-tiles
with U lagging TWO groups behind S: after exp(j)'s sem fires, S(j+2)
outranks U(j) in the Tile scheduler (earlier emission), keeping the exp
stream dense; U's and jobs fill the remaining PE time.  The S->exp->
st-slot-recycle loop is the only tight coupling (st bufs=2).

dtypes: x1/x2/Wq/Wk/Wv and K^T/Q^T/V/E in bf16 (matmul rate identical to
f32r; halves SBUF+DMA so all four x2 slices stay resident for the m-major
K jobs); S stays f32 in PSUM; O^T/Wo/Y^T in f32r.  Measured rel err ~2e-3.

PSUM: st 2x[128,1024] (4 banks) + u 2x[128,512] + jobs 2x[128,512] = 8.
All matmuls 1 row/cycle (moving free dim 512 >= 256).
"""

import numpy as np

# ---- problem constants (hardcoded; kernel.py must be self-contained) ----
B = 4
L = 2048
D = 1024
INNER = 1024
HEADS = 16
DH = 64
N_CORES = 8
IH = INNER // 2  # inner columns per core (8 heads)
SCALE = DH ** -0.5

_CACHE = {}


def _build_nc(L_=L, D_=D, IH_=IH, DH_=DH, LQB=512, XS=512, NOC=64, compile_=True, repeat=1):
    import concourse.bass as bass
    import concourse.tile as tile
    from concourse import bacc, mybir

    f32 = mybir.dt.float32
    f32r = mybir.dt.float32r
    bf16 = mybir.dt.bfloat16
    P = 128
    NH = IH_ // DH_        # heads per core
    NPAIR = NH // 2        # head pairs
    NJT = L_ // P          # lk tiles
    NLQB = L_ // LQB       # lq blocks
    NDT = D_ // P          # D tiles
    NIT = IH_ // P         # inner-half tiles
    NXS = L_ // XS         # x slices
    assert NH % 2 == 0 and L_ % LQB == 0 and LQB % P == 0 and XS == LQB

    # host-packed layouts: one contiguous DMA per weight / per x-slice
    # (HWDGE issue overhead is 625ns per dma_start; 64 tile-DMAs starve the
    # early K jobs).  xh[p, s*D + d*XS + c] = x^T[d*128+p, s*XS+c];
    # wh[p, d*IH + c] = w[d*128+p, c]; woh[p, it*D + c] = wo[it*128+p, c].
    nc = bacc.Bacc("TRN2", target_bir_lowering=False, debug=False)
    x1 = nc.declare_dram_parameter("x1h", [P, L_ * D_ // P], bf16, isOutput=False)
    x2 = nc.declare_dram_parameter("x2h", [P, L_ * D_ // P], bf16, isOutput=False)
    wq = nc.declare_dram_parameter("wqh", [P, D_ * IH_ // P], bf16, isOutput=False)
    wk = nc.declare_dram_parameter("wkh", [P, D_ * IH_ // P], bf16, isOutput=False)
    wv = nc.declare_dram_parameter("wvh", [P, D_ * IH_ // P], bf16, isOutput=False)
    wo = nc.declare_dram_parameter("woh", [P, IH_ * D_ // P], f32r, isOutput=False)
    yt = nc.declare_dram_parameter("yt", [D_, L_], f32, isOutput=True)

    import contextlib

    with tile.TileContext(nc) as tc:
        with (
            tc.For_i(0, repeat, 1) if repeat > 1 else contextlib.nullcontext(),
            tc.tile_pool(name="persist", bufs=1) as persist,
        ):
            # persistent SBUF tensors (one slot each; distinct names)
            kt = persist.tile([P, NIT * L_], bf16, name="kt")    # K^T blocks
            qt = persist.tile([P, NIT * L_], bf16, name="qt")    # Q^T blocks
            vv = persist.tile([P, NJT * NH * (DH_ + NOC)], bf16, name="vv")
            onesrow = persist.tile([P, NH * NOC], f32, name="onesrow")
            wkt = persist.tile([P, NDT * IH_], bf16, name="wkt")
            wvt = persist.tile([P, NDT * IH_], bf16, name="wvt")
            wqt = persist.tile([P, NDT * IH_], bf16, name="wqt")
            wot = persist.tile([P, NIT * D_], f32r, name="wot")
            nc.vector.memset(onesrow, 1.0)
            VJ = NH * (DH_ + NOC)  # per-j-tile v width

            with (
                tc.tile_pool(name="xts2", bufs=1) as xts2,
                tc.tile_pool(name="xts1", bufs=2) as xts1,
                tc.tile_pool(name="jobs", bufs=2, space="PSUM") as jobs,
                tc.tile_pool(name="spsum", bufs=2, space="PSUM") as s_pool,
                tc.tile_pool(name="upsum", bufs=2, space="PSUM") as u_pool,
                tc.tile_pool(name="ets", bufs=6) as ets,
                tc.tile_pool(name="smalls", bufs=1) as smalls,
                tc.tile_pool(name="ot_pool", bufs=2) as ot_pool,
                tc.tile_pool(name="youts", bufs=4) as youts,
            ):
                # ones columns of the V_aug layout (DVE cast-copy to bf16)
                for j in range(NJT):
                    ones_ap = (vv[:, j * VJ:(j + 1) * VJ]
                               .rearrange("p (h c) -> p h c", c=DH_ + NOC)[:, :, DH_:])
                    nc.vector.tensor_copy(ones_ap, onesrow.rearrange(
                        "p (h c) -> p h c", c=NOC))

                # ---- DMA issue: one contiguous transfer per weight/slice,
                # deadline-ordered on the in-order DMA queue ----
                xt2 = {}   # s -> slice tile [P, NDT*XS]; d-tile = col view
                xt1s = {}

                def load_x2_slice(s):
                    t = xts2.tile([P, NDT * XS], bf16, name="x2t",
                                  tag=f"x2_{s}", bufs=1)
                    nc.sync.dma_start(
                        out=t, in_=x2[:, s * NDT * XS:(s + 1) * NDT * XS])
                    xt2[s] = t

                def load_x1_slice(sl):
                    t = xts1.tile([P, NDT * XS], bf16, name="x1t",
                                  tag="x1", bufs=2)
                    nc.sync.dma_start(
                        out=t, in_=x1[:, sl * NDT * XS:(sl + 1) * NDT * XS])
                    xt1s[sl] = t

                # half-transfers up front: Tile's range-based dependency
                # tracking lets the d0-3 matmuls of the first K/Q jobs start
                # after half a slice has landed
                MW = NDT * P  # one m-group of a (m-major) weight
                HX = NDT * XS // 2
                nc.sync.dma_start(out=wkt[:, 0:MW], in_=wk[:, 0:MW])
                t0 = xts2.tile([P, NDT * XS], bf16, name="x2t", tag="x2_0",
                               bufs=1)
                xt2[0] = t0
                nc.sync.dma_start(out=t0[:, 0:HX], in_=x2[:, 0:HX])
                nc.sync.dma_start(out=t0[:, HX:], in_=x2[:, HX:2 * HX])
                nc.sync.dma_start(out=wqt[:, 0:MW], in_=wq[:, 0:MW])
                t1 = xts1.tile([P, NDT * XS], bf16, name="x1t", tag="x1",
                               bufs=2)
                xt1s[0] = t1
                nc.sync.dma_start(out=t1[:, 0:HX], in_=x1[:, 0:HX])
                nc.sync.dma_start(out=t1[:, HX:], in_=x1[:, HX:2 * HX])
                nc.sync.dma_start(out=wkt[:, MW:], in_=wk[:, MW:NIT * MW])
                nc.sync.dma_start(out=wvt, in_=wv[:, :])
                nc.sync.dma_start(out=wqt[:, MW:], in_=wq[:, MW:NIT * MW])
                for s in range(1, NXS):
                    load_x2_slice(s)
                nc.sync.dma_start(out=wot, in_=wo[:, :])

                # ---- uniform single-bank PSUM jobs ----
                def kjob(m, s):
                    t = jobs.tile([P, XS], f32, name="kj", tag="job")
                    for d in range(NDT):
                        nc.tensor.matmul(
                            t,
                            lhsT=wkt[:, (m * NDT + d) * P: (m * NDT + d + 1) * P],
                            rhs=xt2[s][:, d * XS:(d + 1) * XS],
                            start=(d == 0), stop=(d == NDT - 1))
                    nc.vector.tensor_copy(
                        kt[:, m * L_ + s * XS: m * L_ + (s + 1) * XS], t)

                def vjob(s, tq):
                    t = jobs.tile([P, IH_], f32, name="vj", tag="job")
                    for d in range(NDT):
                        nc.tensor.matmul(
                            t,
                            lhsT=xt2[s][:, d * XS + tq * P: d * XS + (tq + 1) * P],
                            rhs=wvt[:, d * IH_:(d + 1) * IH_],
                            start=(d == 0), stop=(d == NDT - 1))
                    j = s * (XS // P) + tq
                    dst = (vv[:, j * VJ:(j + 1) * VJ]
                           .rearrange("p (h c) -> p h c", c=DH_ + NOC)[:, :, :DH_])
                    nc.vector.tensor_copy(dst, t.rearrange("p (h c) -> p h c", c=DH_))

                def qjob(sl, m):
                    t = jobs.tile([P, XS], f32, name="qj", tag="job")
                    for d in range(NDT):
                        nc.tensor.matmul(
                            t,
                            lhsT=wqt[:, (m * NDT + d) * P: (m * NDT + d + 1) * P],
                            rhs=xt1s[sl][:, d * XS:(d + 1) * XS],
                            start=(d == 0), stop=(d == NDT - 1))
                    nc.vector.tensor_copy(
                        qt[:, m * L_ + sl * XS: m * L_ + (sl + 1) * XS], t)

                def yjob(ot_prev, lqb_prev, dsub, py=None, it0=0):
                    if py is None:
                        py = jobs.tile([P, LQB], f32, name="py", tag="job")
                    for it in range(it0, NIT):
                        nc.tensor.matmul(
                            py[:, 0:LQB],
                            lhsT=wot[:, it * D_ + dsub * P: it * D_ + (dsub + 1) * P],
                            rhs=ot_prev[:, it * LQB:(it + 1) * LQB],
                            start=(it == 0), stop=(it == NIT - 1))
                    yo = youts.tile([P, LQB], f32, name="yo")
                    nc.vector.tensor_copy(yo, py[:, 0:LQB])
                    nc.sync.dma_start(
                        out=yt[dsub * P:(dsub + 1) * P,
                               lqb_prev * LQB:(lqb_prev + 1) * LQB],
                        in_=yo)

                def yjob_partial(ot_prev, dsub, pool=None, width=1):
                    # last block: accumulate the pair-0..2 contributions of
                    # Y^T before pair 3's normalization lands
                    if pool is None:
                        pool = jobs
                    py = pool.tile([P, width * LQB], f32, name="pyp",
                                   tag="job" if pool is jobs else "st")
                    for it in range(NIT - 1):
                        nc.tensor.matmul(
                            py[:, 0:LQB],
                            lhsT=wot[:, it * D_ + dsub * P: it * D_ + (dsub + 1) * P],
                            rhs=ot_prev[:, it * LQB:(it + 1) * LQB],
                            start=(it == 0), stop=False)
                    return py

                def run_job(spec):
                    kind = spec[0]
                    if kind == "k":
                        kjob(spec[1], spec[2])
                    elif kind == "v":
                        vjob(spec[1], spec[2])
                    elif kind == "q":
                        qjob(spec[1], spec[2])

                # ---- attention helpers ----
                def emit_norm(hp, u0, u1, ot_b, last=False):
                    for hh, u in ((0, u0), (1, u1)):
                        # all-DVE normalization: recip of the (replicated)
                        # denominator rows, quadrant-broadcast via
                        # stream_shuffle, fused multiply evicts O^T
                        # NOC == DH: the reciprocal of the replicated
                        # denominator rows lines up 1:1 with the V rows --
                        # no broadcast needed
                        rcp = smalls.tile([NOC, LQB], f32, name="rcp",
                                          tag=f"rcp{hh}")
                        nc.vector.reciprocal(rcp, u[DH_:DH_ + NOC, :])
                        dst = ot_b[hh * DH_:(hh + 1) * DH_,
                                   hp * LQB:(hp + 1) * LQB]
                        nc.vector.tensor_mul(dst, u[0:DH_, :], rcp)

                def emit_u(hp, u0, u1, et, j, ot_b):
                    for hh, u in ((0, u0), (1, u1)):
                        h = 2 * hp + hh
                        nc.tensor.matmul(
                            u[0:DH_ + NOC, :],
                            lhsT=vv[:, j * VJ + h * (DH_ + NOC): j * VJ + (h + 1) * (DH_ + NOC)],
                            rhs=et[:, hh * LQB:(hh + 1) * LQB],
                            start=(j == 0), stop=(j == NJT - 1))
                    if j == NJT - 1:
                        emit_norm(hp, u0, u1, ot_b,
                                  last=(hp == NPAIR - 1 and ot_b is last_ot[0]))

                # block-0 insert schedule: all remaining K/V/Q jobs, placed so
                # each pair's S inputs (kt m-band, qt m-group) complete a pair
                # early, and V slices land roughly with their consuming U's
                # EMISSION order is dependency order in Tile: every job must
                # be emitted strictly before its first consumer (vjob(s,t)
                # before the U of j=4s+t, popped at group j//2+2 of pair 0;
                # kjob(m,s) before S(pair m, j=4s); qjob(0,m) before pair m)
                b0_ins = {
                    (0, 0): [("k", 0, 1), ("v", 1, 0)],
                    (0, 1): [("k", 0, 2), ("v", 1, 1)],
                    (0, 2): [("k", 0, 3), ("v", 1, 2), ("q", 0, 1)],
                    (0, 3): [("v", 1, 3), ("v", 2, 0)],
                    (0, 4): [("v", 2, 1), ("v", 2, 2)],
                    (0, 5): [("v", 2, 3), ("v", 3, 0)],
                    (0, 6): [("v", 3, 1), ("v", 3, 2)],
                    (0, 7): [("v", 3, 3), ("k", 1, 0)],
                    (1, 0): [("k", 1, 1), ("k", 1, 2)],
                    (1, 1): [("k", 1, 3)],
                    (1, 2): [("q", 0, 2)],
                    (1, 3): [("k", 2, 0)],
                    (1, 4): [("k", 2, 1), ("k", 2, 2)],
                    (1, 5): [("k", 2, 3)],
                    (2, 0): [("q", 0, 3)],
                    (2, 1): [("k", 3, 0), ("k", 3, 1)],
                    (2, 2): [("k", 3, 2), ("k", 3, 3)],
                    (2, 4): [("q", 1, 0)], (2, 6): [("q", 1, 1)],
                    (3, 1): [("q", 1, 2)], (3, 4): [("q", 1, 3)],
                }

                # prologue: just enough for pair 0's first S/exp/U
                kjob(0, 0)
                qjob(0, 0)
                for tq in range(XS // P):
                    vjob(0, tq)

                ydefer = []  # (ot, lqb, dsub) of the previous block
                pend = []    # (hp, u0, u1, et, j, ot) awaiting the U matmul;
                             # carried across pair/block edges so the last
                             # j-group's U never waits on its exp
                last_ot = [None]
                partials = []

                for lqb in range(NLQB):
                    if lqb < NLQB - 1:
                        load_x1_slice(lqb + 1)
                    ot = ot_pool.tile([P, NIT * LQB], f32r, name="ot")
                    if lqb == NLQB - 1:
                        last_ot[0] = ot
                    for hp in range(NPAIR):
                        u0 = u_pool.tile([P, LQB], f32, name="u0", tag="u")
                        u1 = u_pool.tile([P, LQB], f32, name="u1", tag="u")
                        for g in range(NJT // 2):
                            for jj in (2 * g, 2 * g + 1):
                                st = s_pool.tile([P, 2 * LQB], f32, name="st",
                                                 tag="st")
                                # head pair rides PE rows 0-63 / 64-127
                                nc.tensor.matmul(
                                    st[:, 0:LQB],
                                    lhsT=kt[0:DH_, hp * L_ + jj * P: hp * L_ + (jj + 1) * P],
                                    rhs=qt[0:DH_, hp * L_ + lqb * LQB: hp * L_ + lqb * LQB + LQB],
                                    start=True, stop=True)
                                nc.tensor.matmul(
                                    st[:, LQB:2 * LQB],
                                    lhsT=kt[DH_:2 * DH_, hp * L_ + jj * P: hp * L_ + (jj + 1) * P],
                                    rhs=qt[DH_:2 * DH_, hp * L_ + lqb * LQB: hp * L_ + lqb * LQB + LQB],
                                    start=True, stop=True)
                                et = ets.tile([P, 2 * LQB], bf16, name="et")
                                nc.scalar.activation(
                                    et, st, mybir.ActivationFunctionType.Exp,
                                    scale=float(SCALE))
                                pend.append((hp, u0, u1, et, jj, ot))
                            while len(pend) > 4:  # U lags 2 groups behind S
                                emit_u(*pend.pop(0))
                            if lqb == 0:
                                for spec in b0_ins.get((hp, g), ()):
                                    run_job(spec)
                            else:
                                if g in (2, 6) and ydefer:
                                    yjob(*ydefer.pop(0))
                                if g == 4 and lqb < NLQB - 1:
                                    qjob(lqb + 1, hp)
                                if (lqb == NLQB - 1 and hp == NPAIR - 1
                                        and g == 7):
                                    partials.append((yjob_partial(ot, 0), 0))
                                    partials.append((yjob_partial(ot, 1), 1))
                    skip = {0, 1} if lqb == NLQB - 1 else ()
                    ydefer.extend((ot, lqb, dsub)
                                  for dsub in range(D_ // P) if dsub not in skip)
                while pend:
                    emit_u(*pend.pop(0))
                # two more partials ride the now-idle st banks: their pair-0..2
                # matmuls fill PE during the last normalization's DVE chain
                partials.append((yjob_partial(last_ot[0], 2, s_pool, 2), 2))
                partials.append((yjob_partial(last_ot[0], 3, s_pool, 2), 3))
                ydefer = [t for t in ydefer if t[2] not in (2, 3)]
                # tail: last block's output projections (partials first)
                for py, dsub in partials:
                    yjob(last_ot[0], NLQB - 1, dsub, py=py, it0=NIT - 1)
                while ydefer:
                    yjob(*ydefer.pop(0))
    if compile_:
        nc.compile()
    return nc


def _get_nc():
    if "nc" not in _CACHE:
        _CACHE["nc"] = _build_nc()
    return _CACHE["nc"]


def kernel(x1, x2, Wq, Wkv, Wo, bo):
    import sys
    if "/opt/trn_rl_repo" not in sys.path:
        sys.path.insert(0, "/opt/trn_rl_repo")
    from concourse.bass_utils import run_bass_kernel_spmd

    x1 = np.asarray(x1, dtype=np.float32)
    x2 = np.asarray(x2, dtype=np.float32)
    Wq = np.asarray(Wq, dtype=np.float32)
    Wkv = np.asarray(Wkv, dtype=np.float32)
    Wo = np.asarray(Wo, dtype=np.float32)
    bo = np.asarray(bo, dtype=np.float32)

    nc = _get_nc()
    res = run_bass_kernel_spmd(nc, _make_in_maps(x1, x2, Wq, Wkv, Wo),
                               list(range(N_CORES)))
    return _gather(res.results, bo)


def _pack_x(xt, dt):
    # [D, L] -> [128, NXS * D] with xh[p, s*D + d*XS + c] = xt[d*128+p, s*XS+c]
    NXS, XS, NDT, P = 4, 512, 8, 128  # matches _build_nc defaults
    v = xt.reshape(NDT, P, NXS, XS).transpose(1, 2, 0, 3).reshape(P, NXS * NDT * XS)
    return np.ascontiguousarray(v).astype(dt)


def _pack_w(w, dt):
    # [K, N] -> [128, (K//128) * N] with wh[p, d*N + c] = w[d*128+p, c]
    K, N = w.shape
    v = w.reshape(K // 128, 128, N).transpose(1, 0, 2).reshape(128, (K // 128) * N)
    return np.ascontiguousarray(v).astype(dt)


def _pack_w_mmajor(w, dt):
    # [K, N] -> [128, (K//128) * N] with wh[p, (m*(K//128)+d)*128 + c] =
    # w[d*128+p, m*128+c]: each m-group's weights are contiguous
    K, N = w.shape
    v = (w.reshape(K // 128, 128, N // 128, 128)      # d p m c
         .transpose(1, 2, 0, 3)                        # p m d c
         .reshape(128, (K // 128) * N))
    return np.ascontiguousarray(v).astype(dt)


def _make_in_maps(x1, x2, Wq, Wkv, Wo):
    import ml_dtypes
    bf = ml_dtypes.bfloat16
    x1h = [_pack_x(x1[b].T, bf) for b in range(B)]
    x2h = [_pack_x(x2[b].T, bf) for b in range(B)]
    in_maps = []
    for c in range(N_CORES):
        b, t = c // 2, c % 2
        in_maps.append({
            "x1h": x1h[b],
            "x2h": x2h[b],
            "wqh": _pack_w_mmajor(Wq[:, t * IH:(t + 1) * IH], bf),
            "wkh": _pack_w_mmajor(Wkv[:, t * IH:(t + 1) * IH], bf),
            "wvh": _pack_w(Wkv[:, INNER + t * IH: INNER + (t + 1) * IH], bf),
            "woh": _pack_w(Wo[t * IH:(t + 1) * IH, :], np.float32),
        })
    return in_maps


def _gather(outs, bo):
    y = np.empty((B, L, D), dtype=np.float32)
    for b in range(B):
        y[b] = (outs[2 * b]["yt"] + outs[2 * b + 1]["yt"]).T + bo
    return y


# revision 33
# speedup vs baseline: 1.1673x; 1.0046x over previous
"""Trainium2 Bass kernel for CrossAttention (B=4, L=2048, D=1024, 16 heads x 64).

Sharding: 8 cores = 4 batches x 2 head-halves (tensor parallel over heads:
Wq/Wkv column-split, Wo row-split).  Per core: Q = x1 @ Wq[:, half],
K/V = x2 @ Wkv[:, half-cols], 8 heads of attention, partial Y^T =
Wo[half-rows]^T @ O^T.  The host transposes x per batch (and casts x/Wq/Wkv
to bf16) during sharding, sums the two partial Y^T per batch, transposes
back, adds the bias.

v6 schedule: there is no projection "phase" -- every projection is a
single-PSUM-bank accumulate-and-copy *job* (K per (m-group, x-slice), V per
(x-slice, quarter), Q per (slice, m-group), Y^T per (block, D-subtile)),
and the jobs are woven into the attention group loop.  Attention block 0
absorbs the whole K/V/Q production, so the exp stream on ACT (265.7us busy,
the #2 engine after PE's 327.7us) starts ~10us in instead of ~70us, and the
schedule is PE-bound end to end.  The j-loop runs in groups of 2 j-tiles
with U lagging TWO groups behind S: after exp(j)'s sem fires, S(j+2)
outranks U(j) in the Tile scheduler (earlier emission), keeping the exp
stream dense; U's and jobs fill the remaining PE time.  The S->exp->
st-slot-recycle loop is the only tight coupling (st bufs=2).

dtypes: x1/x2/Wq/Wk/Wv and K^T/Q^T/V/E in bf16 (matmul rate identical to
f32r; halves SBUF+DMA so all four x2 slices stay resident for the m-major
K jobs); S stays f32 in PSUM; O^T/Wo/Y^T in f32r.  Measured rel err ~2e-3.

PSUM: st 2x[128,1024] (4 banks) + u 2x[128,512] + jobs 2x[128,512] = 8.
All matmuls 1 row/cycle (moving free dim 512 >= 256).
"""

import numpy as np

# ---- problem constants (hardcoded; kernel.py must be self-contained) ----
B = 4
L = 2048
D = 1024
INNER = 1024
HEADS = 16
DH = 64
N_CORES = 8
IH = INNER // 2  # inner columns per core (8 heads)
SCALE = DH ** -0.5

_CACHE = {}


def _build_nc(L_=L, D_=D, IH_=IH, DH_=DH, LQB=512, XS=512, NOC=64, compile_=True, repeat=1):
    import concourse.bass as bass
    import concourse.tile as tile
    from concourse import bacc, mybir

    f32 = mybir.dt.float32
    f32r = mybir.dt.float32r
    bf16 = mybir.dt.bfloat16
    P = 128
    NH = IH_ // DH_        # heads per core
    NPAIR = NH // 2        # head pairs
    NJT = L_ // P          # lk tiles
    NLQB = L_ // LQB       # lq blocks
    NDT = D_ // P          # D tiles
    NIT = IH_ // P         # inner-half tiles
    NXS = L_ // XS         # x slices
    assert NH % 2 == 0 and L_ % LQB == 0 and LQB % P == 0 and XS == LQB

    # host-packed layouts: one contiguous DMA per weight / per x-slice
    # (HWDGE issue overhead is 625ns per dma_start; 64 tile-DMAs starve the
    # early K jobs).  xh[p, s*D + d*XS + c] = x^T[d*128+p, s*XS+c];
    # wh[p, d*IH + c] = w[d*128+p, c]; woh[p, it*D + c] = wo[it*128+p, c].
    nc = bacc.Bacc("TRN2", target_bir_lowering=False, debug=False)
    x1 = nc.declare_dram_parameter("x1h", [P, L_ * D_ // P], bf16, isOutput=False)
    x2 = nc.declare_dram_parameter("x2h", [P, L_ * D_ // P], bf16, isOutput=False)
    wq = nc.declare_dram_parameter("wqh", [P, D_ * IH_ // P], bf16, isOutput=False)
    wk = nc.declare_dram_parameter("wkh", [P, D_ * IH_ // P], bf16, isOutput=False)
    wv = nc.declare_dram_parameter("wvh", [P, D_ * IH_ // P], bf16, isOutput=False)
    wo = nc.declare_dram_parameter("woh", [P, IH_ * D_ // P], f32r, isOutput=False)
    yt = nc.declare_dram_parameter("yt", [D_, L_], f32, isOutput=True)

    import contextlib

    with tile.TileContext(nc) as tc:
        with (
            tc.For_i(0, repeat, 1) if repeat > 1 else contextlib.nullcontext(),
            tc.tile_pool(name="persist", bufs=1) as persist,
        ):
            # persistent SBUF tensors (one slot each; distinct names)
            kt = persist.tile([P, NIT * L_], bf16, name="kt")    # K^T blocks
            qt = persist.tile([P, NIT * L_], bf16, name="qt")    # Q^T blocks
            vv = persist.tile([P, NJT * NH * (DH_ + NOC)], bf16, name="vv")
            onesrow = persist.tile([P, NH * NOC], f32, name="onesrow")
            wkt = persist.tile([P, NDT * IH_], bf16, name="wkt")
            wvt = persist.tile([P, NDT * IH_], bf16, name="wvt")
            wqt = persist.tile([P, NDT * IH_], bf16, name="wqt")
            wot = persist.tile([P, NIT * D_], f32r, name="wot")
            nc.vector.memset(onesrow, 1.0)
            VJ = NH * (DH_ + NOC)  # per-j-tile v width

            with (
                tc.tile_pool(name="xts2", bufs=1) as xts2,
                tc.tile_pool(name="xts1", bufs=2) as xts1,
                tc.tile_pool(name="jobs", bufs=2, space="PSUM") as jobs,
                tc.tile_pool(name="spsum", bufs=2, space="PSUM") as s_pool,
                tc.tile_pool(name="upsum", bufs=2, space="PSUM") as u_pool,
                tc.tile_pool(name="ets", bufs=6) as ets,
                tc.tile_pool(name="smalls", bufs=1) as smalls,
                tc.tile_pool(name="ot_pool", bufs=2) as ot_pool,
                tc.tile_pool(name="youts", bufs=4) as youts,
            ):
                # ones columns of the V_aug layout (DVE cast-copy to bf16)
                for j in range(NJT):
                    ones_ap = (vv[:, j * VJ:(j + 1) * VJ]
                               .rearrange("p (h c) -> p h c", c=DH_ + NOC)[:, :, DH_:])
                    nc.vector.tensor_copy(ones_ap, onesrow.rearrange(
                        "p (h c) -> p h c", c=NOC))

                # ---- DMA issue: one contiguous transfer per weight/slice,
                # deadline-ordered on the in-order DMA queue ----
                xt2 = {}   # s -> slice tile [P, NDT*XS]; d-tile = col view
                xt1s = {}

                def load_x2_slice(s):
                    t = xts2.tile([P, NDT * XS], bf16, name="x2t",
                                  tag=f"x2_{s}", bufs=1)
                    nc.sync.dma_start(
                        out=t, in_=x2[:, s * NDT * XS:(s + 1) * NDT * XS])
                    xt2[s] = t

                def load_x1_slice(sl):
                    t = xts1.tile([P, NDT * XS], bf16, name="x1t",
                                  tag="x1", bufs=2)
                    nc.sync.dma_start(
                        out=t, in_=x1[:, sl * NDT * XS:(sl + 1) * NDT * XS])
                    xt1s[sl] = t

                # half-transfers up front: Tile's range-based dependency
                # tracking lets the d0-3 matmuls of the first K/Q jobs start
                # after half a slice has landed
                MW = NDT * P  # one m-group of a (m-major) weight
                HX = NDT * XS // 2
                nc.sync.dma_start(out=wkt[:, 0:MW], in_=wk[:, 0:MW])
                t0 = xts2.tile([P, NDT * XS], bf16, name="x2t", tag="x2_0",
                               bufs=1)
                xt2[0] = t0
                nc.sync.dma_start(out=t0[:, 0:HX], in_=x2[:, 0:HX])
                nc.sync.dma_start(out=t0[:, HX:], in_=x2[:, HX:2 * HX])
                nc.sync.dma_start(out=wqt[:, 0:MW], in_=wq[:, 0:MW])
                t1 = xts1.tile([P, NDT * XS], bf16, name="x1t", tag="x1",
                               bufs=2)
                xt1s[0] = t1
                nc.sync.dma_start(out=t1[:, 0:HX], in_=x1[:, 0:HX])
                nc.sync.dma_start(out=t1[:, HX:], in_=x1[:, HX:2 * HX])
                nc.sync.dma_start(out=wvt, in_=wv[:, :])
                load_x2_slice(1)
                nc.sync.dma_start(out=wqt[:, MW:2 * MW], in_=wq[:, MW:2 * MW])
                nc.sync.dma_start(out=wkt[:, MW:2 * MW], in_=wk[:, MW:2 * MW])
                load_x2_slice(2)
                nc.sync.dma_start(out=wqt[:, 2 * MW:], in_=wq[:, 2 * MW:NIT * MW])
                nc.sync.dma_start(out=wkt[:, 2 * MW:], in_=wk[:, 2 * MW:NIT * MW])
                load_x2_slice(3)
                nc.sync.dma_start(out=wot, in_=wo[:, :])

                # ---- uniform single-bank PSUM jobs ----
                def kjob(m, s):
                    t = jobs.tile([P, XS], f32, name="kj", tag="job")
                    for d in range(NDT):
                        nc.tensor.matmul(
                            t,
                            lhsT=wkt[:, (m * NDT + d) * P: (m * NDT + d + 1) * P],
                            rhs=xt2[s][:, d * XS:(d + 1) * XS],
                            start=(d == 0), stop=(d == NDT - 1))
                    nc.vector.tensor_copy(
                        kt[:, m * L_ + s * XS: m * L_ + (s + 1) * XS], t)

                def vjob(s, tq):
                    t = jobs.tile([P, IH_], f32, name="vj", tag="job")
                    for d in range(NDT):
                        nc.tensor.matmul(
                            t,
                            lhsT=xt2[s][:, d * XS + tq * P: d * XS + (tq + 1) * P],
                            rhs=wvt[:, d * IH_:(d + 1) * IH_],
                            start=(d == 0), stop=(d == NDT - 1))
                    j = s * (XS // P) + tq
                    dst = (vv[:, j * VJ:(j + 1) * VJ]
                           .rearrange("p (h c) -> p h c", c=DH_ + NOC)[:, :, :DH_])
                    nc.vector.tensor_copy(dst, t.rearrange("p (h c) -> p h c", c=DH_))

                def qjob(sl, m):
                    t = jobs.tile([P, XS], f32, name="qj", tag="job")
                    for d in range(NDT):
                        nc.tensor.matmul(
                            t,
                            lhsT=wqt[:, (m * NDT + d) * P: (m * NDT + d + 1) * P],
                            rhs=xt1s[sl][:, d * XS:(d + 1) * XS],
                            start=(d == 0), stop=(d == NDT - 1))
                    nc.vector.tensor_copy(
                        qt[:, m * L_ + sl * XS: m * L_ + (sl + 1) * XS], t)

                def yjob(ot_prev, lqb_prev, dsub, py=None, it0=0):
                    if py is None:
                        py = jobs.tile([P, LQB], f32, name="py", tag="job")
                    for it in range(it0, NIT):
                        nc.tensor.matmul(
                            py[:, 0:LQB],
                            lhsT=wot[:, it * D_ + dsub * P: it * D_ + (dsub + 1) * P],
                            rhs=ot_prev[:, it * LQB:(it + 1) * LQB],
                            start=(it == 0), stop=(it == NIT - 1))
                    yo = youts.tile([P, LQB], f32, name="yo")
                    nc.vector.tensor_copy(yo, py[:, 0:LQB])
                    nc.sync.dma_start(
                        out=yt[dsub * P:(dsub + 1) * P,
                               lqb_prev * LQB:(lqb_prev + 1) * LQB],
                        in_=yo)

                def yjob_partial(ot_prev, dsub, pool=None, width=1):
                    # last block: accumulate the pair-0..2 contributions of
                    # Y^T before pair 3's normalization lands
                    if pool is None:
                        pool = jobs
                    py = pool.tile([P, width * LQB], f32, name="pyp",
                                   tag="job" if pool is jobs else "st")
                    for it in range(NIT - 1):
                        nc.tensor.matmul(
                            py[:, 0:LQB],
                            lhsT=wot[:, it * D_ + dsub * P: it * D_ + (dsub + 1) * P],
                            rhs=ot_prev[:, it * LQB:(it + 1) * LQB],
                            start=(it == 0), stop=False)
                    return py

                def run_job(spec):
                    kind = spec[0]
                    if kind == "k":
                        kjob(spec[1], spec[2])
                    elif kind == "v":
                        vjob(spec[1], spec[2])
                    elif kind == "q":
                        qjob(spec[1], spec[2])

                # ---- attention helpers ----
                def emit_norm(hp, u0, u1, ot_b, last=False):
                    for hh, u in ((0, u0), (1, u1)):
                        # all-DVE normalization: recip of the (replicated)
                        # denominator rows, quadrant-broadcast via
                        # stream_shuffle, fused multiply evicts O^T
                        # NOC == DH: the reciprocal of the replicated
                        # denominator rows lines up 1:1 with the V rows --
                        # no broadcast needed
                        rcp = smalls.tile([NOC, LQB], f32, name="rcp",
                                          tag=f"rcp{hh}")
                        nc.vector.reciprocal(rcp, u[DH_:DH_ + NOC, :])
                        dst = ot_b[hh * DH_:(hh + 1) * DH_,
                                   hp * LQB:(hp + 1) * LQB]
                        nc.vector.tensor_mul(dst, u[0:DH_, :], rcp)

                def emit_u(hp, u0, u1, et, j, ot_b):
                    for hh, u in ((0, u0), (1, u1)):
                        h = 2 * hp + hh
                        nc.tensor.matmul(
                            u[0:DH_ + NOC, :],
                            lhsT=vv[:, j * VJ + h * (DH_ + NOC): j * VJ + (h + 1) * (DH_ + NOC)],
                            rhs=et[:, hh * LQB:(hh + 1) * LQB],
                            start=(j == 0), stop=(j == NJT - 1))
                    if j == NJT - 1:
                        emit_norm(hp, u0, u1, ot_b,
                                  last=(hp == NPAIR - 1 and ot_b is last_ot[0]))

                # block-0 insert schedule: all remaining K/V/Q jobs, placed so
                # each pair's S inputs (kt m-band, qt m-group) complete a pair
                # early, and V slices land roughly with their consuming U's
                # EMISSION order is dependency order in Tile: every job must
                # be emitted strictly before its first consumer (vjob(s,t)
                # before the U of j=4s+t, popped at group j//2+2 of pair 0;
                # kjob(m,s) before S(pair m, j=4s); qjob(0,m) before pair m)
                b0_ins = {
                    (0, 0): [("k", 0, 1), ("v", 1, 0)],
                    (0, 1): [("k", 0, 2), ("v", 1, 1)],
                    (0, 2): [("k", 0, 3), ("v", 1, 2), ("q", 0, 1)],
                    (0, 3): [("v", 1, 3), ("v", 2, 0)],
                    (0, 4): [("v", 2, 1), ("v", 2, 2)],
                    (0, 5): [("v", 2, 3), ("v", 3, 0)],
                    (0, 6): [("v", 3, 1), ("v", 3, 2)],
                    (0, 7): [("v", 3, 3), ("k", 1, 0)],
                    (1, 0): [("k", 1, 1), ("k", 1, 2)],
                    (1, 1): [("k", 1, 3)],
                    (1, 2): [("q", 0, 2)],
                    (1, 3): [("k", 2, 0)],
                    (1, 4): [("k", 2, 1), ("k", 2, 2)],
                    (1, 5): [("k", 2, 3)],
                    (2, 0): [("q", 0, 3)],
                    (2, 1): [("k", 3, 0), ("k", 3, 1)],
                    (2, 2): [("k", 3, 2), ("k", 3, 3)],
                    (2, 4): [("q", 1, 0)], (2, 6): [("q", 1, 1)],
                    (3, 1): [("q", 1, 2)], (3, 4): [("q", 1, 3)],
                }

                # prologue: just enough for pair 0's first S/exp/U
                kjob(0, 0)
                qjob(0, 0)
                for tq in range(XS // P):
                    vjob(0, tq)

                ydefer = []  # (ot, lqb, dsub) of the previous block
                pend = []    # (hp, u0, u1, et, j, ot) awaiting the U matmul;
                             # carried across pair/block edges so the last
                             # j-group's U never waits on its exp
                last_ot = [None]
                partials = []

                for lqb in range(NLQB):
                    if lqb < NLQB - 1:
                        load_x1_slice(lqb + 1)
                    ot = ot_pool.tile([P, NIT * LQB], f32r, name="ot")
                    if lqb == NLQB - 1:
                        last_ot[0] = ot
                    for hp in range(NPAIR):
                        u0 = u_pool.tile([P, LQB], f32, name="u0", tag="u")
                        u1 = u_pool.tile([P, LQB], f32, name="u1", tag="u")
                        for g in range(NJT // 2):
                            for jj in (2 * g, 2 * g + 1):
                                st = s_pool.tile([P, 2 * LQB], f32, name="st",
                                                 tag="st")
                                # head pair rides PE rows 0-63 / 64-127
                                nc.tensor.matmul(
                                    st[:, 0:LQB],
                                    lhsT=kt[0:DH_, hp * L_ + jj * P: hp * L_ + (jj + 1) * P],
                                    rhs=qt[0:DH_, hp * L_ + lqb * LQB: hp * L_ + lqb * LQB + LQB],
                                    start=True, stop=True)
                                nc.tensor.matmul(
                                    st[:, LQB:2 * LQB],
                                    lhsT=kt[DH_:2 * DH_, hp * L_ + jj * P: hp * L_ + (jj + 1) * P],
                                    rhs=qt[DH_:2 * DH_, hp * L_ + lqb * LQB: hp * L_ + lqb * LQB + LQB],
                                    start=True, stop=True)
                                et = ets.tile([P, 2 * LQB], bf16, name="et")
                                nc.scalar.activation(
                                    et, st, mybir.ActivationFunctionType.Exp,
                                    scale=float(SCALE))
                                pend.append((hp, u0, u1, et, jj, ot))
                            while len(pend) > 4:  # U lags 2 groups behind S
                                emit_u(*pend.pop(0))
                            if lqb == 0:
                                for spec in b0_ins.get((hp, g), ()):
                                    run_job(spec)
                            else:
                                if g in (2, 6) and ydefer:
                                    yjob(*ydefer.pop(0))
                                if g == 4 and lqb < NLQB - 1:
                                    qjob(lqb + 1, hp)
                                if (lqb == NLQB - 1 and hp == NPAIR - 1
                                        and g == 7):
                                    partials.append((yjob_partial(ot, 0), 0))
                                    partials.append((yjob_partial(ot, 1), 1))
                    skip = {0, 1} if lqb == NLQB - 1 else ()
                    ydefer.extend((ot, lqb, dsub)
                                  for dsub in range(D_ // P) if dsub not in skip)
                while pend:
                    emit_u(*pend.pop(0))
                # two more partials ride the now-idle st banks: their pair-0..2
                # matmuls fill PE during the last normalization's DVE chain
                partials.append((yjob_partial(last_ot[0], 2, s_pool, 2), 2))
                partials.append((yjob_partial(last_ot[0], 3, s_pool, 2), 3))
                ydefer = [t for t in ydefer if t[2] not in (2, 3)]
                # tail: last block's output projections (partials first)
                for py, dsub in partials:
                    yjob(last_ot[0], NLQB - 1, dsub, py=py, it0=NIT - 1)
                while ydefer:
                    yjob(*ydefer.pop(0))
    if compile_:
        nc.compile()
    return nc


def _get_nc():
    if "nc" not in _CACHE:
        _CACHE["nc"] = _build_nc()
    return _CACHE["nc"]


def kernel(x1, x2, Wq, Wkv, Wo, bo):
    import sys
    if "/opt/trn_rl_repo" not in sys.path:
        sys.path.insert(0, "/opt/trn_rl_repo")
    from concourse.bass_utils import run_bass_kernel_spmd

    x1 = np.asarray(x1, dtype=np.float32)
    x2 = np.asarray(x2, dtype=np.float32)
    Wq = np.asarray(Wq, dtype=np.float32)
    Wkv = np.asarray(Wkv, dtype=np.float32)
    Wo = np.asarray(Wo, dtype=np.float32)
    bo = np.asarray(bo, dtype=np.float32)

    nc = _get_nc()
    res = run_bass_kernel_spmd(nc, _make_in_maps(x1, x2, Wq, Wkv, Wo),
                               list(range(N_CORES)))
    return _gather(res.results, bo)


def _pack_x(xt, dt):
    # [D, L] -> [128, NXS * D] with xh[p, s*D + d*XS + c] = xt[d*128+p, s*XS+c]
    NXS, XS, NDT, P = 4, 512, 8, 128  # matches _build_nc defaults
    v = xt.reshape(NDT, P, NXS, XS).transpose(1, 2, 0, 3).reshape(P, NXS * NDT * XS)
    return np.ascontiguousarray(v).astype(dt)


def _pack_w(w, dt):
    # [K, N] -> [128, (K//128) * N] with wh[p, d*N + c] = w[d*128+p, c]
    K, N = w.shape
    v = w.reshape(K // 128, 128, N).transpose(1, 0, 2).reshape(128, (K // 128) * N)
    return np.ascontiguousarray(v).astype(dt)


def _pack_w_mmajor(w, dt):
    # [K, N] -> [128, (K//128) * N] with wh[p, (m*(K//128)+d)*128 + c] =
    # w[d*128+p, m*128+c]: each m-group's weights are contiguous
    K, N = w.shape
    v = (w.reshape(K // 128, 128, N // 128, 128)      # d p m c
         .transpose(1, 2, 0, 3)                        # p m d c
         .reshape(128, (K // 128) * N))
    return np.ascontiguousarray(v).astype(dt)


def _make_in_maps(x1, x2, Wq, Wkv, Wo):
    import ml_dtypes
    bf = ml_dtypes.bfloat16
    x1h = [_pack_x(x1[b].T, bf) for b in range(B)]
    x2h = [_pack_x(x2[b].T, bf) for b in range(B)]
    in_maps = []
    for c in range(N_CORES):
        b, t = c // 2, c % 2
        in_maps.append({
            "x1h": x1h[b],
            "x2h": x2h[b],
            "wqh": _pack_w_mmajor(Wq[:, t * IH:(t + 1) * IH], bf),
            "wkh": _pack_w_mmajor(Wkv[:, t * IH:(t + 1) * IH], bf),
            "wvh": _pack_w(Wkv[:, INNER + t * IH: INNER + (t + 1) * IH], bf),
            "woh": _pack_w(Wo[t * IH:(t + 1) * IH, :], np.float32),
        })
    return in_maps


def _gather(outs, bo):
    y = np.empty((B, L, D), dtype=np.float32)
    for b in range(B):
        y[b] = (outs[2 * b]["yt"] + outs[2 * b + 1]["yt"]).T + bo
    return y
